# revision 20
# baseline (speedup 1.0000x reference)
import numpy as np
import ml_dtypes

# Problem constants (hardcoded; kernel.py must be self-contained)
N, D, T, K, P = 4000, 256, 52, 20, 100
M = 8            # cores
NS = N // M      # 500 patients per core
KP = 32          # K padded to 32 so each t-group stays inside one partition tile
NBLK = (T * KP) // 128   # 13 blocks of 128 (t,k) rows
DC = 2           # d-chunks of 128
HALF_G = (7, 6)  # 4t-groups per half (7*4=28 t, 6*4=24 t)

BF16 = ml_dtypes.bfloat16


def _make_kernel_mat(length_scale):
    t = np.arange(T, dtype=np.float32)
    sq = (t[None, :] - t[:, None]) ** 2
    Kmat = np.exp(-0.5 * sq / np.float32(length_scale) ** 2).astype(np.float32)
    jitter = 1e-4
    eye = np.eye(T, dtype=np.float32)
    while True:
        if np.linalg.cond(Kmat + jitter * eye) < 1e4:
            break
        jitter *= 2
        if jitter > 0.1:
            break
    return (Kmat + jitter * eye).astype(np.float32)


_KINV_LAM = np.linalg.inv(_make_kernel_mat(T / 4).astype(np.float64))
_KINV_PHI = np.linalg.inv(_make_kernel_mat(T / 3).astype(np.float64))

_COMPILED = {}


def _build_nc():
    import os
    import concourse.bass as bass
    import concourse.mybir as mybir
    from concourse import bacc, tile

    use_paged = os.environ.get("KPAGED", "0") == "1"
    use_fastr = os.environ.get("KFASTR", "1") == "1"
    use_masktt = os.environ.get("KMASKTT", "1") == "1"

    fp32 = mybir.dt.float32
    bf16 = mybir.dt.bfloat16
    Alu = mybir.AluOpType
    Act = mybir.ActivationFunctionType

    nc = bacc.Bacc(None, target_bir_lowering=False)

    # ---- DRAM inputs (host-prepacked layouts) ----
    lam32_d = nc.dram_tensor("lam32", [128, NBLK * NS], bf16, kind="ExternalInput")
    phi32_d = nc.dram_tensor("phi32", [128, NBLK * D], bf16, kind="ExternalInput")
    efT_d = nc.dram_tensor("efT", [128, DC * NS], bf16, kind="ExternalInput")
    yeT_d = nc.dram_tensor("yeT", [128, DC * NS], bf16, kind="ExternalInput")
    # lam rows interleaved with one-hole-per-k for the device-written mean
    lamg_d = nc.dram_tensor("lamg", [125, 4 * K * (T + 1)], bf16,
                            kind="ExternalInput")
    gtg_d = nc.dram_tensor("gtg", [P, NS + K], bf16, kind="ExternalInput")
    # [phi_row | lp_row] pairs for the fused phi gram
    phig2_d = nc.dram_tensor("phig2", [128, 40 * 2 * T], bf16,
                             kind="ExternalInput")
    # event-time one-hot masks (t-major), one 26000-wide block per chunk
    b_d = nc.dram_tensor("bmask", [128, DC * T * NS], bf16, kind="ExternalInput")
    if use_masktt:
        # interleaved [a_h0|b_h0|a_h1|b_h1] per chunk
        masks_d = nc.dram_tensor("masks", [128, DC * 2 * T * NS], bf16,
                                 kind="ExternalInput")
    # scratch for the 1/R broadcast bounce
    rr16_d = nc.dram_tensor("rr16scr", [T, NS], bf16, kind="Internal")

    # ---- DRAM outputs ----
    o_dacc = nc.dram_tensor("o_dacc", [128, 8], fp32, kind="ExternalOutput")
    o_glam = nc.dram_tensor("o_glam", [T + 1, T + 1], fp32, kind="ExternalOutput")
    o_gphi = nc.dram_tensor("o_gphi", [2 * T, 2 * T], fp32, kind="ExternalOutput")

    with tile.TileContext(nc) as tc:
        with (
            tc.tile_pool(name="res", bufs=1) as res,
            tc.tile_pool(name="scr", bufs=4) as scr,
            tc.tile_pool(name="cpx", bufs=2) as cpx,
        ):
            efT = res.tile([128, DC * NS], bf16, tag="efT")
            nc.sync.dma_start(efT[:], efT_d[:])
            yeT = res.tile([128, DC * NS], bf16, tag="yeT")
            nc.sync.dma_start(yeT[:], yeT_d[:])
            theta = res.tile([128, NBLK * NS], bf16, tag="theta")
            phibar = res.tile([128, NBLK * D], bf16, tag="phibar")
            ones32 = res.tile([128, KP], bf16, tag="ones32")
            nc.vector.memset(ones32[:], 1.0)
            dacc = res.tile([128, 8], fp32, tag="dacc")
            nc.vector.memset(dacc[:], 0.0)

            with tc.tile_pool(name="setup", bufs=1) as setup:
                lam32 = setup.tile([128, NBLK * NS], bf16, tag="lam32")
                nc.sync.dma_start(lam32[:], lam32_d[:])
                phi32 = setup.tile([128, NBLK * D], bf16, tag="phi32")
                nc.sync.dma_start(phi32[:], phi32_d[:])
                lamg = setup.tile([125, 4 * K * (T + 1)], bf16, tag="lamg")
                nc.sync.dma_start(lamg[:], lamg_d[:])
                gtg = setup.tile([P, NS + K], bf16, tag="gtg")
                nc.sync.dma_start(gtg[:], gtg_d[:])
                phig2 = setup.tile([128, 40 * 2 * T], bf16, tag="phig2")
                nc.sync.dma_start(phig2[:], phig2_d[:])
                gout = setup.tile([2 * T, 2 * T + T + 1], fp32, tag="gout")

                # ===== GP phase: mean, then fused grams =====
                with tc.tile_pool(name="gps", bufs=1,
                                  space=bass.MemorySpace.PSUM) as gps:
                    mean_ps = gps.tile([125, 512], fp32, tag="mean_ps")
                    for b in range(4):
                        nc.tensor.matmul(mean_ps[:, 0:K],
                                         gtg[:, b * 125:(b + 1) * 125],
                                         gtg[:, NS:NS + K])
                        # write mean into the per-(b,k) hole at slot offset 52
                        hole = lamg[:, :].rearrange(
                            "p (s w) -> p s w", w=T + 1)[:, b * K:(b + 1) * K,
                                                         T:T + 1]
                        nc.scalar.activation(hole, mean_ps[:, 0:K].unsqueeze(2),
                                             Act.Copy)

                    glam_ps = gps.tile([T + 1, 512], fp32, tag="glam_ps")
                    nmm = 4 * K
                    for i in range(nmm):
                        v = lamg[:, i * (T + 1):(i + 1) * (T + 1)]
                        nc.tensor.matmul(glam_ps[:, 0:T + 1], v, v,
                                         start=(i == 0), stop=(i == nmm - 1),
                                         skip_group_check=True)
                    gphi_ps = gps.tile([2 * T, 512], fp32, tag="gphi_ps")
                    for i in range(40):
                        v = phig2[:, i * 2 * T:(i + 1) * 2 * T]
                        nc.tensor.matmul(gphi_ps[:, 0:2 * T], v, v,
                                         start=(i == 0), stop=(i == 39),
                                         skip_group_check=True)
                    nc.scalar.activation(gout[0:T + 1, 0:T + 1],
                                         glam_ps[0:T + 1, 0:T + 1], Act.Copy)
                    nc.scalar.activation(gout[:, T + 1:T + 1 + 2 * T],
                                         gphi_ps[:, 0:2 * T], Act.Copy)
                    nc.sync.dma_start(o_glam[:], gout[0:T + 1, 0:T + 1])
                    nc.sync.dma_start(o_gphi[:], gout[:, T + 1:T + 1 + 2 * T])

                # ===== theta = softmax(lambda): e * (1/R) =====
                e32 = setup.tile([128, NBLK * NS], bf16, tag="e32")
                nc.scalar.activation(e32[:], lam32[:], Act.Exp)
                racc = setup.tile([T, NS], fp32, tag="racc")
                with tc.tile_pool(name="rrp", bufs=2,
                                  space=bass.MemorySpace.PSUM) as rrp:
                    for blk in range(NBLK):
                        rrep = rrp.tile([128, 512], fp32, tag="rrep")
                        for pg in range(4):
                            nc.tensor.matmul(
                                rrep[pg * 32:(pg + 1) * 32, 0:NS],
                                ones32[pg * 32:(pg + 1) * 32, :],
                                e32[pg * 32:pg * 32 + KP,
                                    blk * NS:(blk + 1) * NS],
                                skip_group_check=True,
                                tile_position=(pg * 32, pg * 32))
                        if use_fastr:
                            rc = scr.tile([128, NS], fp32, tag="rc")
                            nc.scalar.activation(rc[:], rrep[:, 0:NS],
                                                 Act.Copy)
                            # gather the 4 unique t-rows {0,32,64,96}
                            src = rc[:, :].rearrange(
                                "(a b) n -> a b n", b=32)[:, 0, :]
                            nc.sync.dma_start(racc[4 * blk:4 * blk + 4, :],
                                              src)
                        else:
                            rinv_b = scr.tile([128, NS], fp32, tag="rinvb")
                            nc.vector.reciprocal(rinv_b[:], rrep[:, 0:NS])
                            nc.vector.tensor_tensor(
                                out=theta[:, blk * NS:(blk + 1) * NS],
                                in0=e32[:, blk * NS:(blk + 1) * NS],
                                in1=rinv_b[:], op=Alu.mult)
                if use_fastr:
                    # one cheap reciprocal on the unique rows, then bf16
                    rinv = setup.tile([T, NS], fp32, tag="rinv")
                    nc.vector.reciprocal(rinv[:], racc[:])
                    rinv16 = setup.tile([T, NS], bf16, tag="rinv16")
                    nc.vector.tensor_copy(rinv16[:], rinv[:])
                    # broadcast back via a DRAM bounce (same SP DMA queue
                    # keeps write->read ordering)
                    nc.sync.dma_start(rr16_d[:], rinv16[:])
                    rrb = setup.tile([128, NBLK * NS], bf16, tag="rrb")
                    rv3 = rr16_d[:].rearrange("(a b) n -> a b n", b=4)
                    for pg in range(4):
                        s = rv3[:, pg, :]
                        s2 = s.unsqueeze(0).broadcast_to([32, NBLK, NS])
                        nc.sync.dma_start(
                            rrb[pg * 32:(pg + 1) * 32, :].rearrange(
                                "p (b n) -> p b n", b=NBLK), s2)
                    nc.vector.tensor_tensor(out=theta[:], in0=e32[:],
                                            in1=rrb[:], op=Alu.mult)

                # phibar = 1 - sigmoid(phi)
                phis = setup.tile([128, NBLK * D], bf16, tag="phis")
                nc.scalar.activation(phis[:], phi32[:], Act.Sigmoid)
                nc.vector.tensor_scalar(out=phibar[:], in0=phis[:],
                                        scalar1=-1.0, scalar2=1.0,
                                        op0=Alu.mult, op1=Alu.add)

            # ===== data-loss main loop =====
            with (
                tc.tile_pool(name="big", bufs=1) as big,
                tc.tile_pool(name="pi4p", bufs=2,
                             space=bass.MemorySpace.PSUM) as pi4p,
            ):
                for c in range(DC):
                    efc = efT[:, c * NS:(c + 1) * NS]
                    L1f = big.tile([128, T * NS], bf16, tag="L1f")
                    if not use_masktt:
                        bm = big.tile([128, T * NS], bf16, tag="bm")
                        nc.sync.dma_start(bm[:],
                                          b_d[:, c * T * NS:(c + 1) * T * NS])
                    phA = big.tile([128, 4 * NS], bf16, tag="phA")
                    nc.vector.memset(phA[:], 0.0)
                    phP = big.tile([128, 4 * NS], bf16, tag="phP")
                    nc.vector.memset(phP[:], 0.0)

                    g0 = 0
                    for h in range(2):
                        ng = HALF_G[h]
                        for g in range(g0, g0 + ng):
                            pi4 = pi4p.tile([128, 4 * 512], fp32, tag="pi4")
                            for j in range(4):
                                t = 4 * g + j
                                blk, prow = t // 4, 32 * (t % 4)
                                nc.tensor.matmul(
                                    pi4[:, j * 512:j * 512 + NS],
                                    phibar[prow:prow + KP,
                                           blk * D + c * 128:
                                           blk * D + c * 128 + 128],
                                    theta[prow:prow + KP,
                                          blk * NS:(blk + 1) * NS],
                                    skip_group_check=True,
                                    tile_position=(prow, 0))
                            pi4v = pi4[:, :].rearrange(
                                "p (t q) -> p t q", q=512)[:, :, 0:NS]
                            nc.scalar.activation(
                                L1f[:, g * 4 * NS:(g + 1) * 4 * NS].rearrange(
                                    "p (t n) -> p t n", t=4),
                                pi4v, Act.Ln)

                        lo, hi = g0 * 4 * NS, (g0 + ng) * 4 * NS
                        nt = ng * 4
                        if use_masktt:
                            moff = c * 2 * T * NS + (0 if h == 0 else
                                                     2 * HALF_G[0] * 4 * NS)
                            hlen = nt * NS
                            am = big.tile([128, HALF_G[0] * 4 * NS], bf16,
                                          tag="mask", bufs=2)
                            nc.sync.dma_start(am[:, 0:hlen],
                                              masks_d[:, moff:moff + hlen])
                            bm2 = big.tile([128, HALF_G[0] * 4 * NS], bf16,
                                           tag="mask", bufs=2)
                            nc.sync.dma_start(
                                bm2[:, 0:hlen],
                                masks_d[:, moff + hlen:moff + 2 * hlen])
                            gw = 4 * NS
                            for g in range(g0, g0 + ng):
                                s0_ = (g - g0) * gw
                                sl_g = slice(g * gw, (g + 1) * gw)
                                sl_m = slice(s0_, s0_ + gw)
                                nc.vector.tensor_tensor(
                                    out=am[:, sl_m], in0=am[:, sl_m],
                                    in1=L1f[:, sl_g], op=Alu.mult)
                                nc.vector.tensor_tensor(
                                    out=phA[:], in0=phA[:],
                                    in1=am[:, sl_m], op=Alu.add)
                                nc.vector.tensor_tensor(
                                    out=bm2[:, sl_m], in0=bm2[:, sl_m],
                                    in1=L1f[:, sl_g], op=Alu.mult)
                                nc.vector.tensor_tensor(
                                    out=phP[:], in0=phP[:],
                                    in1=bm2[:, sl_m], op=Alu.add)
                            g0 += ng
                            continue
                        # p = b * L1  (in place over the mask tile)
                        nc.vector.tensor_tensor(out=bm[:, lo:hi],
                                                in0=bm[:, lo:hi],
                                                in1=L1f[:, lo:hi], op=Alu.mult)
                        if use_paged:
                            # psi = (t <= ef) * L1  (in place over L1f)
                            lv = L1f[:, lo:hi].rearrange(
                                "p (t n) -> p t n", t=nt)
                            mo = efc.unsqueeze(1).broadcast_to([128, nt, NS])
                            nc.vector.tensor_paged_mask(
                                out=lv, in_=lv,
                                partition_indices=float(4 * g0 - 1),
                                partition_step=1.0, mask_offsets=mo)
                        else:
                            for t in range(4 * g0, 4 * (g0 + ng)):
                                sl_ = slice(t * NS, (t + 1) * NS)
                                nc.vector.scalar_tensor_tensor(
                                    out=L1f[:, sl_], in0=efc,
                                    scalar=float(t), in1=L1f[:, sl_],
                                    op0=Alu.is_ge, op1=Alu.mult)
                        # phase folds
                        for g in range(g0, g0 + ng):
                            s = slice(g * 4 * NS, (g + 1) * 4 * NS)
                            nc.vector.tensor_tensor(out=phA[:], in0=phA[:],
                                                    in1=L1f[:, s], op=Alu.add)
                            nc.vector.tensor_tensor(out=phP[:], in0=phP[:],
                                                    in1=bm[:, s], op=Alu.add)
                        g0 += ng

                    # ---- finals: fold 4 phases -> [128, NS] ----
                    psA = cpx.tile([128, NS], bf16, tag="psA")
                    nc.vector.tensor_tensor(out=psA[:], in0=phA[:, 0:NS],
                                            in1=phA[:, NS:2 * NS], op=Alu.add)
                    nc.vector.tensor_tensor(out=psA[:], in0=psA[:],
                                            in1=phA[:, 2 * NS:3 * NS],
                                            op=Alu.add)
                    nc.vector.tensor_tensor(out=psA[:], in0=psA[:],
                                            in1=phA[:, 3 * NS:4 * NS],
                                            op=Alu.add)
                    ce = cpx.tile([128, NS], bf16, tag="ce")
                    nc.vector.tensor_tensor(out=ce[:], in0=phP[:, 0:NS],
                                            in1=phP[:, NS:2 * NS], op=Alu.add)
                    nc.vector.tensor_tensor(out=ce[:], in0=ce[:],
                                            in1=phP[:, 2 * NS:3 * NS],
                                            op=Alu.add)
                    nc.vector.tensor_tensor(out=ce[:], in0=ce[:],
                                            in1=phP[:, 3 * NS:4 * NS],
                                            op=Alu.add)
                    # dacc col c = sum_n sum_t psi
                    nc.vector.tensor_reduce(out=dacc[:, c:c + 1], in_=psA[:],
                                            axis=mybir.AxisListType.X,
                                            op=Alu.add)
                    # ---- event correction ----
                    X = cpx.tile([128, NS], fp32, tag="X")
                    nc.scalar.activation(X[:], ce[:], Act.Exp)
                    yec = yeT[:, c * NS:(c + 1) * NS]
                    gt_ = cpx.tile([128, NS], fp32, tag="g")
                    nc.vector.tensor_tensor(out=gt_[:], in0=X[:], in1=yec,
                                            op=Alu.add)
                    nc.vector.tensor_scalar(out=gt_[:], in0=gt_[:],
                                            scalar1=-1.0, scalar2=2.0,
                                            op0=Alu.mult, op1=Alu.add)
                    nc.vector.tensor_scalar(out=gt_[:], in0=gt_[:],
                                            scalar1=1e-9, scalar2=None,
                                            op0=Alu.max)
                    lnG = cpx.tile([128, NS], fp32, tag="lnG")
                    nc.scalar.activation(lnG[:], gt_[:], Act.Ln)
                    nc.vector.tensor_tensor(out=lnG[:], in0=lnG[:], in1=ce[:],
                                            op=Alu.subtract)
                    nc.vector.scalar_tensor_tensor(
                        out=lnG[:], in0=yec, scalar=1.0, in1=lnG[:],
                        op0=Alu.mult, op1=Alu.mult,
                        accum_out=dacc[:, 2 + c: 3 + c])

            nc.sync.dma_start(o_dacc[:], dacc[:])

    if not nc.is_finalized():
        nc.finalize()
    return nc


def _prep_inputs(lambda_, phi, gamma, G, Y, logit_prev_t, event_times):
    lam = np.asarray(lambda_, dtype=np.float32)
    phi = np.asarray(phi, dtype=np.float32)
    gamma = np.asarray(gamma, dtype=np.float32)
    G = np.asarray(G, dtype=np.float32)
    ef = np.asarray(event_times)

    # phi in (t,k)-packed layout [52,32,256] -> [128, 13*256]
    arrp = np.zeros((T, KP, D), np.float32)
    arrp[:, :K, :] = phi.transpose(2, 0, 1)
    phi32 = np.ascontiguousarray(
        arrp.reshape(NBLK, 128, D).transpose(1, 0, 2).reshape(128, NBLK * D)
    ).astype(BF16)

    # fused phi gram input: [phi_row | lp_row] pairs
    prows = phi.reshape(K * D, T)
    lp_rows = np.tile(np.asarray(logit_prev_t, np.float32), (K, 1))
    pair = np.concatenate([prows, lp_rows], axis=1)          # [5120, 104]
    phig2 = np.ascontiguousarray(
        pair.reshape(40, 128, 2 * T).transpose(1, 0, 2).reshape(128, 40 * 2 * T)
    ).astype(BF16)

    gam16 = gamma.astype(BF16)
    tgrid = np.arange(T, dtype=np.float32)

    in_maps = []
    for c in range(M):
        sl = slice(c * NS, (c + 1) * NS)
        lam_c = lam[sl]                       # [500, 20, 52]
        arr = np.full((T, KP, NS), -1e4, np.float32)
        arr[:, :K, :] = lam_c.transpose(2, 1, 0)
        lam32 = np.ascontiguousarray(
            arr.reshape(NBLK, 128, NS).transpose(1, 0, 2)
            .reshape(128, NBLK * NS)).astype(BF16)

        efc = ef[sl].astype(np.float32)       # [500, 256]
        efp = efc.T.reshape(DC, 128, NS)      # [2, 128, 500] (chunk, d, n)
        efT = np.ascontiguousarray(
            efp.transpose(1, 0, 2).reshape(128, DC * NS)).astype(BF16)
        ye = np.take_along_axis(np.asarray(Y[sl], np.float32),
                                ef[sl][:, :, None].astype(np.int64),
                                axis=2)[:, :, 0]
        yeT = np.ascontiguousarray(
            ye.T.reshape(DC, 128, NS).transpose(1, 0, 2).reshape(128, DC * NS)
        ).astype(BF16)

        # one-hot b mask, t-major per chunk: [128, (chunk, t, n)]
        efr = efp.transpose(1, 0, 2)              # [128, chunk, n]
        bm = (efr[:, :, None, :] == tgrid[None, None, :, None])
        b_host = np.ascontiguousarray(
            bm.reshape(128, DC * T * NS)).astype(BF16)
        am_full = (efr[:, :, None, :] >= tgrid[None, None, :, None])
        # interleave [a_h0|b_h0|a_h1|b_h1] per chunk
        h0 = HALF_G[0] * 4
        parts = []
        for cc in range(DC):
            parts += [am_full[:, cc, :h0, :].reshape(128, -1),
                      bm[:, cc, :h0, :].reshape(128, -1),
                      am_full[:, cc, h0:, :].reshape(128, -1),
                      bm[:, cc, h0:, :].reshape(128, -1)]
        masks_host = np.ascontiguousarray(
            np.concatenate(parts, axis=1)).astype(BF16)

        # lam rows with a mean hole: [125, (4*K slots) x 53]
        lamr = lam_c.reshape(4, 125, K, T).transpose(1, 0, 2, 3)  # [125,4,20,52]
        lamh = np.zeros((125, 4, K, T + 1), np.float32)
        lamh[:, :, :, :T] = lamr
        lamg = np.ascontiguousarray(
            lamh.reshape(125, 4 * K * (T + 1))).astype(BF16)
        gtg = np.concatenate([G[sl].T.astype(BF16), gam16], axis=1)

        im = dict(
            lam32=lam32, phi32=phi32, efT=efT, yeT=yeT, lamg=lamg,
            gtg=np.ascontiguousarray(gtg), phig2=phig2, bmask=b_host,
        )
        import os as _os
        if _os.environ.get("KMASKTT", "1") == "1":
            im["masks"] = masks_host
        in_maps.append(im)
    return in_maps


def kernel(lambda_, phi, gamma, G, Y, logit_prev_t, event_times):
    from concourse.bass_utils import run_bass_kernel_spmd

    if "nc" not in _COMPILED:
        _COMPILED["nc"] = _build_nc()
    nc = _COMPILED["nc"]

    in_maps = _prep_inputs(lambda_, phi, gamma, G, Y, logit_prev_t, event_times)
    res = run_bass_kernel_spmd(nc, in_maps, list(range(M)))

    data_sum = 0.0
    q_lam = 0.0
    for c in range(M):
        r = res.results[c]
        data_sum += float(r["o_dacc"].astype(np.float64).sum())
        g = r["o_glam"].astype(np.float64)
        A = g[0:T, 0:T]
        b = g[T, 0:T].reshape(T, 1)
        m2 = g[T, T]
        ones = np.ones((T, 1))
        S = A - b @ ones.T - ones @ b.T + m2
        q_lam += float((_KINV_LAM * S).sum())
    gp = res.results[0]["o_gphi"].astype(np.float64)
    Ap = gp[0:T, 0:T]
    Bp = gp[T:2 * T, 0:T]
    Cp = gp[T:2 * T, T:2 * T]
    Sp = Ap - Bp - Bp.T + Cp
    q_phi = float((_KINV_PHI * Sp).sum())

    loss = -data_sum / N + 0.5 * q_lam / N + 0.5 * q_phi / D
    return np.array(loss, dtype=np.float32)


# revision 22
# speedup vs baseline: 1.0144x; 1.0144x over previous
import numpy as np
import ml_dtypes

# Problem constants (hardcoded; kernel.py must be self-contained)
N, D, T, K, P = 4000, 256, 52, 20, 100
M = 8            # cores
NS = N // M      # 500 patients per core
KP = 32          # K padded to 32 so each t-group stays inside one partition tile
NBLK = (T * KP) // 128   # 13 blocks of 128 (t,k) rows
DC = 2           # d-chunks of 128
HALF_G = (7, 6)  # 4t-groups per half (7*4=28 t, 6*4=24 t)

BF16 = ml_dtypes.bfloat16


def _make_kernel_mat(length_scale):
    t = np.arange(T, dtype=np.float32)
    sq = (t[None, :] - t[:, None]) ** 2
    Kmat = np.exp(-0.5 * sq / np.float32(length_scale) ** 2).astype(np.float32)
    jitter = 1e-4
    eye = np.eye(T, dtype=np.float32)
    while True:
        if np.linalg.cond(Kmat + jitter * eye) < 1e4:
            break
        jitter *= 2
        if jitter > 0.1:
            break
    return (Kmat + jitter * eye).astype(np.float32)


_KINV_LAM = np.linalg.inv(_make_kernel_mat(T / 4).astype(np.float64))
_KINV_PHI = np.linalg.inv(_make_kernel_mat(T / 3).astype(np.float64))

_COMPILED = {}


def _build_nc():
    import os
    import concourse.bass as bass
    import concourse.mybir as mybir
    from concourse import bacc, tile

    use_paged = os.environ.get("KPAGED", "0") == "1"
    use_fastr = os.environ.get("KFASTR", "1") == "1"
    use_masktt = os.environ.get("KMASKTT", "1") == "1"

    fp32 = mybir.dt.float32
    bf16 = mybir.dt.bfloat16
    Alu = mybir.AluOpType
    Act = mybir.ActivationFunctionType

    nc = bacc.Bacc(None, target_bir_lowering=False)

    # ---- DRAM inputs (host-prepacked layouts) ----
    lam32_d = nc.dram_tensor("lam32", [128, NBLK * NS], bf16, kind="ExternalInput")
    phi32_d = nc.dram_tensor("phi32", [128, NBLK * D], bf16, kind="ExternalInput")
    efT_d = nc.dram_tensor("efT", [128, DC * NS], bf16, kind="ExternalInput")
    yeT_d = nc.dram_tensor("yeT", [128, DC * NS], bf16, kind="ExternalInput")
    # lam rows interleaved with one-hole-per-k for the device-written mean
    lamg_d = nc.dram_tensor("lamg", [125, 4 * K * (T + 1)], bf16,
                            kind="ExternalInput")
    gtg_d = nc.dram_tensor("gtg", [P, NS + K], bf16, kind="ExternalInput")
    # [phi_row | lp_row] pairs for the fused phi gram
    phig2_d = nc.dram_tensor("phig2", [128, 40 * 2 * T], bf16,
                             kind="ExternalInput")
    # event-time one-hot masks (t-major), one 26000-wide block per chunk
    b_d = nc.dram_tensor("bmask", [128, DC * T * NS], bf16, kind="ExternalInput")
    if use_masktt:
        # interleaved [a_h0|b_h0|a_h1|b_h1] per chunk
        masks_d = nc.dram_tensor("masks", [128, DC * 2 * T * NS], bf16,
                                 kind="ExternalInput")
    # scratch for the 1/R broadcast bounce
    rr16_d = nc.dram_tensor("rr16scr", [T, NS], bf16, kind="Internal")

    # ---- DRAM outputs ----
    o_dacc = nc.dram_tensor("o_dacc", [128, 8], fp32, kind="ExternalOutput")
    o_glam = nc.dram_tensor("o_glam", [T + 1, T + 1], fp32, kind="ExternalOutput")
    o_gphi = nc.dram_tensor("o_gphi", [2 * T, 2 * T], fp32, kind="ExternalOutput")

    with tile.TileContext(nc) as tc:
        with (
            tc.tile_pool(name="res", bufs=1) as res,
            tc.tile_pool(name="scr", bufs=4) as scr,
            tc.tile_pool(name="cpx", bufs=2) as cpx,
        ):
            efT = res.tile([128, DC * NS], bf16, tag="efT")
            nc.sync.dma_start(efT[:], efT_d[:])
            yeT = res.tile([128, DC * NS], bf16, tag="yeT")
            nc.sync.dma_start(yeT[:], yeT_d[:])
            theta = res.tile([128, NBLK * NS], bf16, tag="theta")
            phibar = res.tile([128, NBLK * D], bf16, tag="phibar")
            ones32 = res.tile([128, KP], bf16, tag="ones32")
            nc.vector.memset(ones32[:], 1.0)
            dacc = res.tile([128, 8], fp32, tag="dacc")
            nc.vector.memset(dacc[:], 0.0)

            with tc.tile_pool(name="setup", bufs=1) as setup:
                lam32 = setup.tile([128, NBLK * NS], bf16, tag="lam32")
                nc.sync.dma_start(lam32[:], lam32_d[:])
                phi32 = setup.tile([128, NBLK * D], bf16, tag="phi32")
                nc.sync.dma_start(phi32[:], phi32_d[:])
                lamg = setup.tile([125, 4 * K * (T + 1)], bf16, tag="lamg")
                nc.sync.dma_start(lamg[:], lamg_d[:])
                gtg = setup.tile([P, NS + K], bf16, tag="gtg")
                nc.sync.dma_start(gtg[:], gtg_d[:])
                phig2 = setup.tile([128, 40 * 2 * T], bf16, tag="phig2")
                nc.sync.dma_start(phig2[:], phig2_d[:])
                gout = setup.tile([2 * T, 2 * T + T + 1], fp32, tag="gout")

                # ===== GP phase: mean, then fused grams =====
                with tc.tile_pool(name="gps", bufs=1,
                                  space=bass.MemorySpace.PSUM) as gps:
                    mean_ps = gps.tile([125, 512], fp32, tag="mean_ps")
                    for b in range(4):
                        nc.tensor.matmul(mean_ps[:, 0:K],
                                         gtg[:, b * 125:(b + 1) * 125],
                                         gtg[:, NS:NS + K])
                        # write mean into the per-(b,k) hole at slot offset 52
                        hole = lamg[:, :].rearrange(
                            "p (s w) -> p s w", w=T + 1)[:, b * K:(b + 1) * K,
                                                         T:T + 1]
                        nc.scalar.activation(hole, mean_ps[:, 0:K].unsqueeze(2),
                                             Act.Copy)

                    glam_ps = gps.tile([T + 1, 512], fp32, tag="glam_ps")
                    nmm = 4 * K
                    for i in range(nmm):
                        v = lamg[:, i * (T + 1):(i + 1) * (T + 1)]
                        nc.tensor.matmul(glam_ps[:, 0:T + 1], v, v,
                                         start=(i == 0), stop=(i == nmm - 1),
                                         skip_group_check=True)
                    gphi_ps = gps.tile([2 * T, 512], fp32, tag="gphi_ps")
                    for i in range(40):
                        v = phig2[:, i * 2 * T:(i + 1) * 2 * T]
                        nc.tensor.matmul(gphi_ps[:, 0:2 * T], v, v,
                                         start=(i == 0), stop=(i == 39),
                                         skip_group_check=True)
                    nc.scalar.activation(gout[0:T + 1, 0:T + 1],
                                         glam_ps[0:T + 1, 0:T + 1], Act.Copy)
                    nc.scalar.activation(gout[:, T + 1:T + 1 + 2 * T],
                                         gphi_ps[:, 0:2 * T], Act.Copy)
                    nc.sync.dma_start(o_glam[:], gout[0:T + 1, 0:T + 1])
                    nc.sync.dma_start(o_gphi[:], gout[:, T + 1:T + 1 + 2 * T])

                # ===== theta = softmax(lambda): e * (1/R) =====
                e32 = setup.tile([128, NBLK * NS], bf16, tag="e32")
                nc.scalar.activation(e32[:], lam32[:], Act.Exp)
                racc = setup.tile([T, NS], fp32, tag="racc")
                rinv = setup.tile([T, NS], fp32, tag="rinv")
                rinv16 = setup.tile([T, NS], bf16, tag="rinv16")
                rrb = setup.tile([128, NBLK * NS], bf16, tag="rrb")
                BH = (8, 5)
                with tc.tile_pool(name="rrp", bufs=2,
                                  space=bass.MemorySpace.PSUM) as rrp:
                    b0 = 0
                    for rh in range(2):
                        nb = BH[rh]
                        for blk in range(b0, b0 + nb):
                            rrep = rrp.tile([128, 512], fp32, tag="rrep")
                            for pg in range(4):
                                nc.tensor.matmul(
                                    rrep[pg * 32:(pg + 1) * 32, 0:NS],
                                    ones32[pg * 32:(pg + 1) * 32, :],
                                    e32[pg * 32:pg * 32 + KP,
                                        blk * NS:(blk + 1) * NS],
                                    skip_group_check=True,
                                    tile_position=(pg * 32, pg * 32))
                            if use_fastr:
                                rc = scr.tile([128, NS], fp32, tag="rc")
                                nc.scalar.activation(rc[:], rrep[:, 0:NS],
                                                     Act.Copy)
                                src = rc[:, :].rearrange(
                                    "(a b) n -> a b n", b=32)[:, 0, :]
                                nc.sync.dma_start(
                                    racc[4 * blk:4 * blk + 4, :], src)
                            else:
                                rinv_b = scr.tile([128, NS], fp32,
                                                  tag="rinvb")
                                nc.vector.reciprocal(rinv_b[:],
                                                     rrep[:, 0:NS])
                                nc.vector.tensor_tensor(
                                    out=theta[:, blk * NS:(blk + 1) * NS],
                                    in0=e32[:, blk * NS:(blk + 1) * NS],
                                    in1=rinv_b[:], op=Alu.mult)
                        if use_fastr:
                            # finalize this half: recip, bf16, bounce, mult
                            rs = slice(4 * b0, 4 * (b0 + nb))
                            nc.vector.reciprocal(rinv[rs, :], racc[rs, :])
                            nc.vector.tensor_copy(rinv16[rs, :], rinv[rs, :])
                            nc.sync.dma_start(rr16_d[rs, :], rinv16[rs, :])
                            rv3 = rr16_d[:].rearrange(
                                "(a b) n -> a b n", b=4)
                            for pg in range(4):
                                s = rv3[b0:b0 + nb, pg, :]
                                s2 = s.unsqueeze(0).broadcast_to(
                                    [32, nb, NS])
                                nc.sync.dma_start(
                                    rrb[pg * 32:(pg + 1) * 32,
                                        b0 * NS:(b0 + nb) * NS].rearrange(
                                        "p (b n) -> p b n", b=nb), s2)
                            nc.vector.tensor_tensor(
                                out=theta[:, b0 * NS:(b0 + nb) * NS],
                                in0=e32[:, b0 * NS:(b0 + nb) * NS],
                                in1=rrb[:, b0 * NS:(b0 + nb) * NS],
                                op=Alu.mult)
                        b0 += nb

                # phibar = 1 - sigmoid(phi)
                phis = setup.tile([128, NBLK * D], bf16, tag="phis")
                nc.scalar.activation(phis[:], phi32[:], Act.Sigmoid)
                nc.vector.tensor_scalar(out=phibar[:], in0=phis[:],
                                        scalar1=-1.0, scalar2=1.0,
                                        op0=Alu.mult, op1=Alu.add)

            # ===== data-loss main loop =====
            with (
                tc.tile_pool(name="big", bufs=1) as big,
                tc.tile_pool(name="pi4p", bufs=2,
                             space=bass.MemorySpace.PSUM) as pi4p,
            ):
                for c in range(DC):
                    efc = efT[:, c * NS:(c + 1) * NS]
                    L1f = big.tile([128, T * NS], bf16, tag="L1f")
                    if not use_masktt:
                        bm = big.tile([128, T * NS], bf16, tag="bm")
                        nc.sync.dma_start(bm[:],
                                          b_d[:, c * T * NS:(c + 1) * T * NS])
                    phA = big.tile([128, 4 * NS], bf16, tag="phA")
                    nc.vector.memset(phA[:], 0.0)
                    phP = big.tile([128, 4 * NS], bf16, tag="phP")
                    nc.vector.memset(phP[:], 0.0)

                    g0 = 0
                    for h in range(2):
                        ng = HALF_G[h]
                        for g in range(g0, g0 + ng):
                            pi4 = pi4p.tile([128, 4 * 512], fp32, tag="pi4")
                            for j in range(4):
                                t = 4 * g + j
                                blk, prow = t // 4, 32 * (t % 4)
                                nc.tensor.matmul(
                                    pi4[:, j * 512:j * 512 + NS],
                                    phibar[prow:prow + KP,
                                           blk * D + c * 128:
                                           blk * D + c * 128 + 128],
                                    theta[prow:prow + KP,
                                          blk * NS:(blk + 1) * NS],
                                    skip_group_check=True,
                                    tile_position=(prow, 0))
                            pi4v = pi4[:, :].rearrange(
                                "p (t q) -> p t q", q=512)[:, :, 0:NS]
                            nc.scalar.activation(
                                L1f[:, g * 4 * NS:(g + 1) * 4 * NS].rearrange(
                                    "p (t n) -> p t n", t=4),
                                pi4v, Act.Ln)

                        lo, hi = g0 * 4 * NS, (g0 + ng) * 4 * NS
                        nt = ng * 4
                        if use_masktt:
                            moff = c * 2 * T * NS + (0 if h == 0 else
                                                     2 * HALF_G[0] * 4 * NS)
                            hlen = nt * NS
                            am = big.tile([128, HALF_G[0] * 4 * NS], bf16,
                                          tag="mask", bufs=2)
                            nc.sync.dma_start(am[:, 0:hlen],
                                              masks_d[:, moff:moff + hlen])
                            bm2 = big.tile([128, HALF_G[0] * 4 * NS], bf16,
                                           tag="mask", bufs=2)
                            nc.sync.dma_start(
                                bm2[:, 0:hlen],
                                masks_d[:, moff + hlen:moff + 2 * hlen])
                            gw = 4 * NS
                            for g in range(g0, g0 + ng):
                                s0_ = (g - g0) * gw
                                sl_g = slice(g * gw, (g + 1) * gw)
                                sl_m = slice(s0_, s0_ + gw)
                                nc.vector.tensor_tensor(
                                    out=am[:, sl_m], in0=am[:, sl_m],
                                    in1=L1f[:, sl_g], op=Alu.mult)
                                nc.vector.tensor_tensor(
                                    out=phA[:], in0=phA[:],
                                    in1=am[:, sl_m], op=Alu.add)
                                nc.vector.tensor_tensor(
                                    out=bm2[:, sl_m], in0=bm2[:, sl_m],
                                    in1=L1f[:, sl_g], op=Alu.mult)
                                nc.vector.tensor_tensor(
                                    out=phP[:], in0=phP[:],
                                    in1=bm2[:, sl_m], op=Alu.add)
                            g0 += ng
                            continue
                        # p = b * L1  (in place over the mask tile)
                        nc.vector.tensor_tensor(out=bm[:, lo:hi],
                                                in0=bm[:, lo:hi],
                                                in1=L1f[:, lo:hi], op=Alu.mult)
                        if use_paged:
                            # psi = (t <= ef) * L1  (in place over L1f)
                            lv = L1f[:, lo:hi].rearrange(
                                "p (t n) -> p t n", t=nt)
                            mo = efc.unsqueeze(1).broadcast_to([128, nt, NS])
                            nc.vector.tensor_paged_mask(
                                out=lv, in_=lv,
                                partition_indices=float(4 * g0 - 1),
                                partition_step=1.0, mask_offsets=mo)
                        else:
                            for t in range(4 * g0, 4 * (g0 + ng)):
                                sl_ = slice(t * NS, (t + 1) * NS)
                                nc.vector.scalar_tensor_tensor(
                                    out=L1f[:, sl_], in0=efc,
                                    scalar=float(t), in1=L1f[:, sl_],
                                    op0=Alu.is_ge, op1=Alu.mult)
                        # phase folds
                        for g in range(g0, g0 + ng):
                            s = slice(g * 4 * NS, (g + 1) * 4 * NS)
                            nc.vector.tensor_tensor(out=phA[:], in0=phA[:],
                                                    in1=L1f[:, s], op=Alu.add)
                            nc.vector.tensor_tensor(out=phP[:], in0=phP[:],
                                                    in1=bm[:, s], op=Alu.add)
                        g0 += ng

                    # ---- finals: fold 4 phases -> [128, NS] ----
                    psA = cpx.tile([128, NS], bf16, tag="psA")
                    nc.vector.tensor_tensor(out=psA[:], in0=phA[:, 0:NS],
                                            in1=phA[:, NS:2 * NS], op=Alu.add)
                    nc.vector.tensor_tensor(out=psA[:], in0=psA[:],
                                            in1=phA[:, 2 * NS:3 * NS],
                                            op=Alu.add)
                    nc.vector.tensor_tensor(out=psA[:], in0=psA[:],
                                            in1=phA[:, 3 * NS:4 * NS],
                                            op=Alu.add)
                    ce = cpx.tile([128, NS], bf16, tag="ce")
                    nc.vector.tensor_tensor(out=ce[:], in0=phP[:, 0:NS],
                                            in1=phP[:, NS:2 * NS], op=Alu.add)
                    nc.vector.tensor_tensor(out=ce[:], in0=ce[:],
                                            in1=phP[:, 2 * NS:3 * NS],
                                            op=Alu.add)
                    nc.vector.tensor_tensor(out=ce[:], in0=ce[:],
                                            in1=phP[:, 3 * NS:4 * NS],
                                            op=Alu.add)
                    # dacc col c = sum_n sum_t psi
                    nc.vector.tensor_reduce(out=dacc[:, c:c + 1], in_=psA[:],
                                            axis=mybir.AxisListType.X,
                                            op=Alu.add)
                    # ---- event correction ----
                    X = cpx.tile([128, NS], fp32, tag="X")
                    nc.scalar.activation(X[:], ce[:], Act.Exp)
                    yec = yeT[:, c * NS:(c + 1) * NS]
                    gt_ = cpx.tile([128, NS], fp32, tag="g")
                    nc.vector.tensor_tensor(out=gt_[:], in0=X[:], in1=yec,
                                            op=Alu.add)
                    nc.vector.tensor_scalar(out=gt_[:], in0=gt_[:],
                                            scalar1=-1.0, scalar2=2.0,
                                            op0=Alu.mult, op1=Alu.add)
                    nc.vector.tensor_scalar(out=gt_[:], in0=gt_[:],
                                            scalar1=1e-9, scalar2=None,
                                            op0=Alu.max)
                    lnG = cpx.tile([128, NS], fp32, tag="lnG")
                    nc.scalar.activation(lnG[:], gt_[:], Act.Ln)
                    nc.vector.tensor_tensor(out=lnG[:], in0=lnG[:], in1=ce[:],
                                            op=Alu.subtract)
                    nc.vector.scalar_tensor_tensor(
                        out=lnG[:], in0=yec, scalar=1.0, in1=lnG[:],
                        op0=Alu.mult, op1=Alu.mult,
                        accum_out=dacc[:, 2 + c: 3 + c])

            nc.sync.dma_start(o_dacc[:], dacc[:])

    if not nc.is_finalized():
        nc.finalize()
    return nc


def _prep_inputs(lambda_, phi, gamma, G, Y, logit_prev_t, event_times):
    lam = np.asarray(lambda_, dtype=np.float32)
    phi = np.asarray(phi, dtype=np.float32)
    gamma = np.asarray(gamma, dtype=np.float32)
    G = np.asarray(G, dtype=np.float32)
    ef = np.asarray(event_times)

    # phi in (t,k)-packed layout [52,32,256] -> [128, 13*256]
    arrp = np.zeros((T, KP, D), np.float32)
    arrp[:, :K, :] = phi.transpose(2, 0, 1)
    phi32 = np.ascontiguousarray(
        arrp.reshape(NBLK, 128, D).transpose(1, 0, 2).reshape(128, NBLK * D)
    ).astype(BF16)

    # fused phi gram input: [phi_row | lp_row] pairs
    prows = phi.reshape(K * D, T)
    lp_rows = np.tile(np.asarray(logit_prev_t, np.float32), (K, 1))
    pair = np.concatenate([prows, lp_rows], axis=1)          # [5120, 104]
    phig2 = np.ascontiguousarray(
        pair.reshape(40, 128, 2 * T).transpose(1, 0, 2).reshape(128, 40 * 2 * T)
    ).astype(BF16)

    gam16 = gamma.astype(BF16)
    tgrid = np.arange(T, dtype=np.float32)

    in_maps = []
    for c in range(M):
        sl = slice(c * NS, (c + 1) * NS)
        lam_c = lam[sl]                       # [500, 20, 52]
        arr = np.full((T, KP, NS), -1e4, np.float32)
        arr[:, :K, :] = lam_c.transpose(2, 1, 0)
        lam32 = np.ascontiguousarray(
            arr.reshape(NBLK, 128, NS).transpose(1, 0, 2)
            .reshape(128, NBLK * NS)).astype(BF16)

        efc = ef[sl].astype(np.float32)       # [500, 256]
        efp = efc.T.reshape(DC, 128, NS)      # [2, 128, 500] (chunk, d, n)
        efT = np.ascontiguousarray(
            efp.transpose(1, 0, 2).reshape(128, DC * NS)).astype(BF16)
        ye = np.take_along_axis(np.asarray(Y[sl], np.float32),
                                ef[sl][:, :, None].astype(np.int64),
                                axis=2)[:, :, 0]
        yeT = np.ascontiguousarray(
            ye.T.reshape(DC, 128, NS).transpose(1, 0, 2).reshape(128, DC * NS)
        ).astype(BF16)

        # one-hot b mask, t-major per chunk: [128, (chunk, t, n)]
        efr = efp.transpose(1, 0, 2)              # [128, chunk, n]
        bm = (efr[:, :, None, :] == tgrid[None, None, :, None])
        b_host = np.ascontiguousarray(
            bm.reshape(128, DC * T * NS)).astype(BF16)
        am_full = (efr[:, :, None, :] >= tgrid[None, None, :, None])
        # interleave [a_h0|b_h0|a_h1|b_h1] per chunk
        h0 = HALF_G[0] * 4
        parts = []
        for cc in range(DC):
            parts += [am_full[:, cc, :h0, :].reshape(128, -1),
                      bm[:, cc, :h0, :].reshape(128, -1),
                      am_full[:, cc, h0:, :].reshape(128, -1),
                      bm[:, cc, h0:, :].reshape(128, -1)]
        masks_host = np.ascontiguousarray(
            np.concatenate(parts, axis=1)).astype(BF16)

        # lam rows with a mean hole: [125, (4*K slots) x 53]
        lamr = lam_c.reshape(4, 125, K, T).transpose(1, 0, 2, 3)  # [125,4,20,52]
        lamh = np.zeros((125, 4, K, T + 1), np.float32)
        lamh[:, :, :, :T] = lamr
        lamg = np.ascontiguousarray(
            lamh.reshape(125, 4 * K * (T + 1))).astype(BF16)
        gtg = np.concatenate([G[sl].T.astype(BF16), gam16], axis=1)

        im = dict(
            lam32=lam32, phi32=phi32, efT=efT, yeT=yeT, lamg=lamg,
            gtg=np.ascontiguousarray(gtg), phig2=phig2, bmask=b_host,
        )
        import os as _os
        if _os.environ.get("KMASKTT", "1") == "1":
            im["masks"] = masks_host
        in_maps.append(im)
    return in_maps


def kernel(lambda_, phi, gamma, G, Y, logit_prev_t, event_times):
    from concourse.bass_utils import run_bass_kernel_spmd

    if "nc" not in _COMPILED:
        _COMPILED["nc"] = _build_nc()
    nc = _COMPILED["nc"]

    in_maps = _prep_inputs(lambda_, phi, gamma, G, Y, logit_prev_t, event_times)
    res = run_bass_kernel_spmd(nc, in_maps, list(range(M)))

    data_sum = 0.0
    q_lam = 0.0
    for c in range(M):
        r = res.results[c]
        data_sum += float(r["o_dacc"].astype(np.float64).sum())
        g = r["o_glam"].astype(np.float64)
        A = g[0:T, 0:T]
        b = g[T, 0:T].reshape(T, 1)
        m2 = g[T, T]
        ones = np.ones((T, 1))
        S = A - b @ ones.T - ones @ b.T + m2
        q_lam += float((_KINV_LAM * S).sum())
    gp = res.results[0]["o_gphi"].astype(np.float64)
    Ap = gp[0:T, 0:T]
    Bp = gp[T:2 * T, 0:T]
    Cp = gp[T:2 * T, T:2 * T]
    Sp = Ap - Bp - Bp.T + Cp
    q_phi = float((_KINV_PHI * Sp).sum())

    loss = -data_sum / N + 0.5 * q_lam / N + 0.5 * q_phi / D
    return np.array(loss, dtype=np.float32)
